# revision 35
# baseline (speedup 1.0000x reference)
"""MANN cell kernel for 8 TRN2 NeuronCores (nn_MANNCell_90434831385056).

Strategy:
 - LSTM-over-batch scan (shared-state, sequential in the reference) is solved
   with a Picard fixed-point iteration: NSWEEP batched sweeps of
   G = X + Hshift @ W_hh^T + elementwise, replicated on every core.
 - Memory ops are data-parallel over batch: each core handles 16 batches.
   reads = (w_r . erase) @ M_prev + (w_r @ w_w^T) @ k, so M is never
   materialized; a single pass over M_prev (sent as row-normalized bf16 in
   both [n,d] and [d,n] layouts) computes cosine scores, softmax (unshifted:
   cosines are bounded), and the read accumulations.
 - Least-used / erase masks come from value thresholds (4th/1st smallest of
   w_u per row via max8 of -w_u), no argsort needed.
"""
import os
import numpy as np

B, H, N, D, R = 128, 512, 2048, 256, 4
NC = 8
BS = B // NC  # 16 batches per core
NT = N // 128  # 16 n-tiles
NSWEEP = 10

_LAST_RESULTS = {}


def _bf16(x):
    import ml_dtypes
    return np.asarray(x, np.float32).astype(ml_dtypes.bfloat16)


def _build_nc(has_rv, stages=7):
    import concourse.bass as bass
    import concourse.tile as tile
    from concourse import bacc, mybir
    from concourse.masks import make_identity
    from contextlib import ExitStack

    f32 = mybir.dt.float32
    bf16 = mybir.dt.bfloat16
    AF = mybir.ActivationFunctionType
    OP = mybir.AluOpType

    nc = bacc.Bacc(None, target_bir_lowering=False, debug=False)

    xin_d = nc.dram_tensor("xin", [128, 512], f32, kind="ExternalInput")
    h0t_d = nc.dram_tensor("h0t", [128, 4], f32, kind="ExternalInput")
    c0_d = nc.dram_tensor("c0", [1, 512], f32, kind="ExternalInput")
    b2_d = nc.dram_tensor("b2", [128, 2048], f32, kind="ExternalInput")
    bp_d = nc.dram_tensor("bp", [128, 1028], f32, kind="ExternalInput")
    wihT_d = nc.dram_tensor("wihT", [512, 2048], f32, kind="ExternalInput")
    whhT_d = nc.dram_tensor("whhT", [512, 2048], f32, kind="ExternalInput")
    wpT_d = nc.dram_tensor("wpT", [512, 1028], f32, kind="ExternalInput")
    bselT_d = nc.dram_tensor("bselT", [128, BS], f32, kind="ExternalInput")
    mnat_d = nc.dram_tensor("mnat", [128, BS, NT, 256], bf16, kind="ExternalInput")
    mT_d = nc.dram_tensor("mT", [2, 128, BS, 2048], bf16, kind="ExternalInput")
    wuT_d = nc.dram_tensor("wuT", [128, BS, NT], f32, kind="ExternalInput")
    normT_d = nc.dram_tensor("normT", [128, BS, NT], f32, kind="ExternalInput")
    wrpT_d = nc.dram_tensor("wrpT", [128, BS, NT, 4], f32, kind="ExternalInput")
    wu_d = nc.dram_tensor("wu", [BS, 2048], f32, kind="ExternalInput")
    if has_rv:
        xrv_d = nc.dram_tensor("xrv", [128, 2048], f32, kind="ExternalInput")
    out_d = nc.dram_tensor("out", [BS, 1536], f32, kind="ExternalOutput")

    with tile.TileContext(nc) as tc, ExitStack() as ctx:
        P = ctx.enter_context(tc.tile_pool(name="persist", bufs=1))
        mpool = ctx.enter_context(tc.tile_pool(name="mtiles", bufs=2))
        fpool = ctx.enter_context(tc.tile_pool(name="flash", bufs=2))

        ident = P.tile([128, 128], f32)
        make_identity(nc, ident)

        # ---- resident weights / inputs ----
        wihT_sb = P.tile([128, 4, 2048], f32)
        nc.sync.dma_start(out=wihT_sb, in_=wihT_d[:, :].rearrange("(a p) n -> p a n", p=128))
        whhT_sb = P.tile([128, 4, 2048], f32)
        nc.sync.dma_start(out=whhT_sb, in_=whhT_d[:, :].rearrange("(a p) n -> p a n", p=128))
        wpT_sb = P.tile([128, 4, 1028], f32)
        nc.sync.dma_start(out=wpT_sb, in_=wpT_d[:, :].rearrange("(a p) n -> p a n", p=128))
        xin_sb = P.tile([128, 512], f32)
        nc.sync.dma_start(out=xin_sb, in_=xin_d[:, :])
        b2_sb = P.tile([128, 2048], f32)
        nc.sync.dma_start(out=b2_sb, in_=b2_d[:, :])
        bp_sb = P.tile([128, 1028], f32)
        nc.sync.dma_start(out=bp_sb, in_=bp_d[:, :])
        bselT_sb = P.tile([128, BS], f32)
        nc.sync.dma_start(out=bselT_sb, in_=bselT_d[:, :])
        wuT_sb = P.tile([128, BS, NT], f32)
        nc.sync.dma_start(out=wuT_sb, in_=wuT_d[:, :, :])
        normT_sb = P.tile([128, BS, NT], f32)
        nc.sync.dma_start(out=normT_sb, in_=normT_d[:, :, :])
        wrpT_sb = P.tile([128, BS, NT, 4], f32)
        nc.sync.dma_start(out=wrpT_sb, in_=wrpT_d[:, :, :, :])
        wu_sb = P.tile([BS, 2048], f32)
        nc.sync.dma_start(out=wu_sb, in_=wu_d[:, :])
        if has_rv:
            xrv_sb = P.tile([128, 2048], f32)
            nc.sync.dma_start(out=xrv_sb, in_=xrv_d[:, :])

        # ---- w_u thresholds (independent of LSTM) ----
        if stages < 1:
            nc.sync.dma_start(out=out_d[:, :][:, 0:512], in_=wu_sb[:, 0:512])
            return nc
        nc.vector.tensor_scalar_mul(wu_sb, wu_sb, -1.0)
        vals8 = P.tile([BS, 8], f32)
        nc.vector.max(out=vals8, in_=wu_sb)
        th2 = P.tile([BS, 2], f32)
        nc.vector.tensor_scalar_mul(th2[:, 0:1], vals8[:, 0:1], -1.0)
        nc.vector.tensor_scalar_mul(th2[:, 1:2], vals8[:, 3:4], -1.0)
        th1_128 = P.tile([128, BS], f32)
        th4_128 = P.tile([128, BS], f32)

        # ---- X = inputs @ W_ih[:, :512]^T + b2 (+ rv part) ----
        with tc.tile_pool(name="psum_big", bufs=1, space="PSUM") as PSB, \
             tc.tile_pool(name="psum_sm", bufs=2, space="PSUM") as PSS, \
             tc.tile_pool(name="psum_csh", bufs=1, space="PSUM") as PSC:
            ones1 = P.tile([1, 128], f32)
            nc.vector.memset(ones1, 1.0)
            # shift matrix: S[t', t] = 1 iff t == t' + 1 (for c_{t-1} shift)
            shmat = P.tile([128, 128], f32)
            nc.gpsimd.memset(shmat, 0.0)
            nc.gpsimd.affine_select(
                out=shmat, in_=shmat, compare_op=OP.not_equal, fill=1.0,
                base=1, pattern=[[-1, 128]], channel_multiplier=1)
            throw_sb = P.tile([1, 2, BS], f32)
            for j in range(2):
                rp = PSS.tile([1, BS], f32, tag="tp")
                nc.tensor.transpose(rp, th2[:, j:j + 1], ident[0:BS, 0:BS])
                nc.vector.tensor_copy(throw_sb[0:1, j], rp)
            for j, dst in ((0, th1_128), (1, th4_128)):
                bc_p = PSS.tile([128, BS], f32, tag="tp")
                nc.tensor.matmul(bc_p, ones1, throw_sb[0:1, j],
                                 start=True, stop=True)
                nc.vector.tensor_copy(dst, bc_p)

            if stages < 2:
                nc.sync.dma_start(out=out_d[:, :][:, 0:512], in_=wu_sb[:, 0:512])
                return nc
            xinT_sb = P.tile([128, 4, 128], f32)
            for j in range(4):
                pt = PSS.tile([128, 128], f32, tag="tp")
                nc.tensor.transpose(pt, xin_sb[:, j * 128:(j + 1) * 128], ident)
                nc.vector.tensor_copy(xinT_sb[:, j], pt)
            gpsum = PSB.tile([128, 4, 512], f32, tag="big")
            for nch in range(4):
                for kt in range(4):
                    nc.tensor.matmul(
                        gpsum[:, nch], xinT_sb[:, kt],
                        wihT_sb[:, kt, nch * 512:(nch + 1) * 512],
                        start=(kt == 0), stop=(kt == 3))
            X_sb = P.tile([128, 2048], f32)
            nc.vector.scalar_tensor_tensor(
                out=X_sb, in0=gpsum.rearrange("p a n -> p (a n)"), scalar=1.0,
                in1=b2_sb, op0=OP.mult, op1=OP.add)
            if has_rv:
                nc.vector.tensor_add(X_sb, X_sb, xrv_sb)

            if stages < 3:
                nc.sync.dma_start(out=out_d[:, :][:, 0:512], in_=X_sb[:BS, 0:512])
                return nc
            # ---- Picard sweeps ----
            h0t_sb = P.tile([128, 4], f32)
            nc.sync.dma_start(out=h0t_sb, in_=h0t_d[:, :])
            hshiftT = P.tile([128, 4, 128], f32)
            nc.vector.memset(hshiftT, 0.0)
            for j in range(4):
                nc.vector.tensor_copy(hshiftT[:, j, 0:1], h0t_sb[:, j:j + 1])
            c0_sb = P.tile([1, 512], f32)
            nc.sync.dma_start(out=c0_sb, in_=c0_d[:, :])
            cshift = P.tile([128, 512], f32)
            nc.vector.memset(cshift, 0.0)
            e0row = P.tile([1, 128], f32)
            nc.vector.tensor_copy(e0row, ident[0:1, :])

            h_sb = P.tile([128, 512], f32)
            c_sb = P.tile([128, 512], f32)
            act_sb = P.tile([128, 2048], f32)
            prod_sb = P.tile([128, 512], f32)
            tc_sb = P.tile([128, 512], f32)
            hT_final = P.tile([128, 4, 128], f32)

            for s in range(NSWEEP):
                gp = PSB.tile([128, 4, 512], f32, tag="big")
                for nch in range(4):
                    for kt in range(4):
                        nc.tensor.matmul(
                            gp[:, nch], hshiftT[:, kt],
                            whhT_sb[:, kt, nch * 512:(nch + 1) * 512],
                            start=(kt == 0), stop=(kt == 3))
                nc.vector.tensor_add(
                    act_sb, gp.rearrange("p a n -> p (a n)"), X_sb)
                nc.scalar.activation(act_sb[:, 0:1024], act_sb[:, 0:1024],
                                     AF.Sigmoid)
                nc.scalar.activation(act_sb[:, 1536:2048], act_sb[:, 1536:2048],
                                     AF.Sigmoid)
                nc.scalar.activation(act_sb[:, 1024:1536], act_sb[:, 1024:1536],
                                     AF.Tanh)
                nc.vector.tensor_mul(prod_sb, act_sb[:, 0:512],
                                     act_sb[:, 1024:1536])
                nc.vector.tensor_mul(c_sb, act_sb[:, 512:1024], cshift)
                nc.vector.tensor_add(c_sb, c_sb, prod_sb)
                nc.scalar.activation(tc_sb, c_sb, AF.Tanh)
                nc.vector.tensor_mul(h_sb, act_sb[:, 1536:2048], tc_sb)
                if s < NSWEEP - 1:
                    csh_p = PSC.tile([128, 512], f32, tag="csh")
                    nc.tensor.matmul(csh_p, shmat, c_sb, start=True, stop=False)
                    nc.tensor.matmul(csh_p, e0row, c0_sb, start=False, stop=True)
                    nc.vector.tensor_copy(cshift, csh_p)
                    for j in range(4):
                        pt = PSS.tile([128, 128], f32, tag="tp")
                        nc.tensor.transpose(
                            pt, h_sb[:, j * 128:(j + 1) * 128], ident)
                        nc.vector.tensor_copy(
                            hshiftT[:, j, 1:128], pt[:, 0:127])
                else:
                    for j in range(4):
                        pt = PSS.tile([128, 128], f32, tag="tp")
                        nc.tensor.transpose(
                            pt, h_sb[:, j * 128:(j + 1) * 128], ident)
                        nc.vector.tensor_copy(hT_final[:, j], pt)

            # ---- ctrl_out shard -> output ----
            hsh_p = PSS.tile([BS, 512], f32, tag="tp")
            nc.tensor.matmul(hsh_p, bselT_sb, h_sb, start=True, stop=True)
            hshard = P.tile([BS, 512], f32)
            nc.vector.tensor_copy(hshard, hsh_p)
            nc.sync.dma_start(out=out_d[:, :][:, 0:512], in_=hshard)

            if stages < 4:
                return nc
            # ---- params = ctrl_out @ W_p^T + b_p, sharded ----
            ppsum = PSB.tile([128, 4, 512], f32, tag="big")
            chunks = [(0, 512), (512, 512), (1024, 4)]
            for nch, (off, w) in enumerate(chunks):
                for kt in range(4):
                    nc.tensor.matmul(
                        ppsum[:, nch, 0:w], hT_final[:, kt],
                        wpT_sb[:, kt, off:off + w],
                        start=(kt == 0), stop=(kt == 3))
            params_sb = P.tile([128, 1028], f32)
            for nch, (off, w) in enumerate(chunks):
                nc.vector.scalar_tensor_tensor(
                    out=params_sb[:, off:off + w], in0=ppsum[:, nch, 0:w],
                    scalar=1.0,
                    in1=bp_sb[:, off:off + w],
                    op0=OP.mult, op1=OP.add)
            pshard = P.tile([BS, 1028], f32)
            for nch, (off, w) in enumerate(chunks):
                psh_p = PSS.tile([BS, 512], f32, tag="tp")
                nc.tensor.matmul(psh_p[:, 0:w], bselT_sb,
                                 params_sb[:, off:off + w],
                                 start=True, stop=True)
                nc.vector.tensor_copy(pshard[:, off:off + w], psh_p[:, 0:w])

            # ---- k, alpha, kT ----
            k_sb = P.tile([BS, 4, 256], f32)
            for r in range(4):
                nc.scalar.activation(k_sb[:, r], pshard[:, r * 257:r * 257 + 256],
                                     AF.Tanh)
            alpha_sb = P.tile([BS, 4], f32)
            nc.scalar.activation(
                alpha_sb, bass.AP(tensor=pshard.tensor, offset=pshard.offset + 256,
                                  ap=[pshard.ap[0], [257, 4]]),
                AF.Sigmoid)
            al1m_sb = P.tile([BS, 4], f32)
            nc.vector.tensor_scalar(al1m_sb, alpha_sb, -1.0, 1.0,
                                    op0=OP.mult, op1=OP.add)
            alrow_sb = P.tile([1, 8, BS], f32)
            for r in range(4):
                rp1 = PSS.tile([1, BS], f32, tag="tp")
                nc.tensor.transpose(rp1, alpha_sb[:, r:r + 1], ident[0:BS, 0:BS])
                nc.vector.tensor_copy(alrow_sb[0:1, r], rp1)
                rp2 = PSS.tile([1, BS], f32, tag="tp")
                nc.tensor.transpose(rp2, al1m_sb[:, r:r + 1], ident[0:BS, 0:BS])
                nc.vector.tensor_copy(alrow_sb[0:1, 4 + r], rp2)
            alpha128 = P.tile([128, 4, BS], f32)
            al1m128 = P.tile([128, 4, BS], f32)
            for r in range(4):
                bc_p = PSS.tile([128, BS], f32, tag="tp")
                nc.tensor.matmul(bc_p, ones1, alrow_sb[0:1, r],
                                 start=True, stop=True)
                nc.vector.tensor_copy(alpha128[:, r], bc_p)
                bc_p2 = PSS.tile([128, BS], f32, tag="tp")
                nc.tensor.matmul(bc_p2, ones1, alrow_sb[0:1, 4 + r],
                                 start=True, stop=True)
                nc.vector.tensor_copy(al1m128[:, r], bc_p2)
            ksq = P.tile([BS, 4, 256], f32)
            nc.vector.tensor_mul(ksq, k_sb, k_sb)
            knsq = P.tile([BS, 4], f32)
            nc.vector.reduce_sum(knsq, ksq, axis=mybir.AxisListType.X)
            kn_sb = P.tile([BS, 4], f32)
            nc.scalar.activation(kn_sb, knsq, AF.Sqrt)
            rkn_sb = P.tile([BS, 4], f32)
            nc.vector.reciprocal(rkn_sb, kn_sb)
            ksc = P.tile([BS, 4, 256], f32)
            nc.vector.tensor_mul(
                ksc, k_sb,
                bass.AP(tensor=rkn_sb.tensor, offset=rkn_sb.offset,
                        ap=[rkn_sb.ap[0], [1, 4], [0, 256]]))
            if stages < 5:
                return nc
            kTs = P.tile([128, 2, 4, BS], bf16)   # [d, dh, r, t]
            kTraw = P.tile([128, 2, 4, BS], f32)
            for r in range(4):
                for dh in range(2):
                    pt = PSS.tile([128, BS], f32, tag="tp")
                    nc.tensor.transpose(
                        pt, ksc[:, r, dh * 128:(dh + 1) * 128], ident[0:BS, 0:BS])
                    nc.vector.tensor_copy(kTs[:, dh, r], pt)
                    pt2 = PSS.tile([128, BS], f32, tag="tp")
                    nc.tensor.transpose(
                        pt2, k_sb[:, r, dh * 128:(dh + 1) * 128], ident[0:BS, 0:BS])
                    nc.vector.tensor_copy(kTraw[:, dh, r], pt2)

        if stages < 6:
            return nc
        # ---- flash pass over 16 batches ----
        with tc.tile_pool(name="ps_st", bufs=2, space="PSUM") as PST, \
             tc.tile_pool(name="ps_s1", bufs=2, space="PSUM") as PS1, \
             tc.tile_pool(name="ps_r", bufs=2, space="PSUM") as PSR, \
             tc.tile_pool(name="ps_kb", bufs=1, space="PSUM") as PKB:
            for b in range(BS):
                mnat_b = mpool.tile([128, NT, 256], bf16, tag="mnat")
                nc.sync.dma_start(out=mnat_b, in_=mnat_d[:, :, :, :][:, b])
                mT_b = mpool.tile([128, 2, 2048], bf16, tag="mT")
                nc.sync.dma_start(out=mT_b[:, 0], in_=mT_d[:, :, :, :][0, :, b])
                nc.sync.dma_start(out=mT_b[:, 1], in_=mT_d[:, :, :, :][1, :, b])

                # scores^T tiles: [n(128), nt, r]
                stp = PST.tile([128, NT, 4], f32, tag="st")
                for nt in range(NT):
                    for dh in range(2):
                        nc.tensor.matmul(
                            stp[:, nt],
                            mT_b[:, dh, nt * 128:(nt + 1) * 128],
                            kTs[:, dh, :, b],
                            start=(dh == 0), stop=(dh == 1))
                eT = fpool.tile([128, NT, 4], f32, tag="eT")
                nc.scalar.activation(eT, stp, AF.Exp)

                # masks + scales
                keep = fpool.tile([128, NT], f32, tag="keep")
                nc.vector.tensor_scalar(
                    keep, wuT_sb[:, b], th1_128[:, b:b + 1],
                    None, op0=OP.is_gt)
                wlu = fpool.tile([128, NT], f32, tag="wlu")
                nc.vector.tensor_scalar(
                    wlu, wuT_sb[:, b], th4_128[:, b:b + 1],
                    None, op0=OP.is_le)
                scl = fpool.tile([128, NT], f32, tag="scl")
                nc.vector.tensor_mul(scl, normT_sb[:, b], keep)
                eTs = fpool.tile([128, NT, 4], bf16, tag="eTs")
                nc.vector.tensor_mul(
                    eTs, eT,
                    bass.AP(tensor=scl.tensor, offset=scl.offset,
                            ap=[scl.ap[0], [1, NT], [0, 4]]))

                # w_wT with ones column: [n, nt, 5]
                wwT = fpool.tile([128, NT, 5], f32, tag="wwT")
                a_sl = alpha128[:, :, b]
                nc.vector.tensor_mul(
                    wwT[:, :, 0:4], wrpT_sb[:, b],
                    bass.AP(tensor=a_sl.tensor, offset=a_sl.offset,
                            ap=[a_sl.ap[0], [0, NT], [BS, 4]]))
                luax = fpool.tile([128, NT, 4], f32, tag="luax")
                m_sl = al1m128[:, :, b]
                nc.vector.tensor_mul(
                    luax,
                    bass.AP(tensor=wlu.tensor, offset=wlu.offset,
                            ap=[wlu.ap[0], [1, NT], [0, 4]]),
                    bass.AP(tensor=m_sl.tensor, offset=m_sl.offset,
                            ap=[m_sl.ap[0], [0, NT], [BS, 4]]))
                nc.vector.tensor_add(wwT[:, :, 0:4], wwT[:, :, 0:4], luax)
                nc.vector.memset(wwT[:, :, 4:5], 1.0)

                # S1|Z and read accumulation
                s1p = PS1.tile([4, 5], f32, tag="s1")
                for nt in range(NT):
                    nc.tensor.matmul(s1p, eT[:, nt], wwT[:, nt],
                                     start=(nt == 0), stop=(nt == NT - 1))
                s1_sb = fpool.tile([4, 5], f32, tag="s1sb")
                nc.vector.tensor_copy(s1_sb, s1p)
                s1tp = PKB.tile([4, 4], f32, tag="s1t")
                nc.tensor.transpose(s1tp, s1_sb[:, 0:4], ident[0:4, 0:4])
                s1t_sb = fpool.tile([4, 4], f32, tag="s1tsb")
                nc.vector.tensor_copy(s1t_sb, s1tp)
                kbp = PKB.tile([4, 256], f32, tag="kb")
                for dh in range(2):
                    nc.tensor.transpose(
                        kbp[:, dh * 128:(dh + 1) * 128],
                        kTraw[:, dh, :, b],
                        ident)
                kb_sb = fpool.tile([4, 256], f32, tag="kbsb")
                nc.vector.tensor_copy(kb_sb, kbp)

                rp = PSR.tile([4, 256], f32, tag="rd")
                for nt in range(NT):
                    nc.tensor.matmul(rp, eTs[:, nt], mnat_b[:, nt],
                                     start=(nt == 0), stop=False)
                nc.tensor.matmul(rp, s1t_sb, kb_sb, start=False, stop=True)

                rz = fpool.tile([4, 1], f32, tag="rz")
                nc.vector.reciprocal(rz, s1_sb[:, 4:5])
                rd_sb = fpool.tile([4, 256], f32, tag="rdsb")
                nc.vector.tensor_scalar_mul(rd_sb, rp, rz)
                nc.sync.dma_start(
                    out=out_d[:, :][b:b + 1, 512:1536]
                    .rearrange("o (r d) -> (o r) d", r=4),
                    in_=rd_sb)

    return nc


def _ensure_ntff_hook():
    """The container's antenv lacks axon_hooks; shim it so trace=True can
    drive NTFF profiling through libaxon_pjrt's C ABI."""
    try:
        from antenv.axon_hooks import get_axon_ntff_profile_hook
        if get_axon_ntff_profile_hook() is not None:
            return True
    except ImportError:
        pass
    try:
        import sys
        import types
        import antenv
        from trn_agent_boot.trn_boot import _ntff_profile_via_ctypes
        hook = _ntff_profile_via_ctypes('/opt/axon/libaxon_pjrt.so')
        mod = types.ModuleType("antenv.axon_hooks")
        _state = {"h": hook}
        mod.set_axon_ntff_profile_hook = lambda h: _state.update(h=h)
        mod.get_axon_ntff_profile_hook = lambda: _state["h"]
        sys.modules["antenv.axon_hooks"] = mod
        antenv.axon_hooks = mod
        return True
    except Exception:
        return False


def kernel(inputs, h0, c0, read_vectors, w_r_prev, w_u_prev, M_prev,
           W_ih, W_hh, b_ih, b_hh, W_p, b_p):
    from concourse.bass_utils import run_bass_kernel_spmd

    f32 = np.float32
    inputs = np.asarray(inputs, f32)
    M_prev = np.asarray(M_prev, f32)
    w_u_prev = np.asarray(w_u_prev, f32)
    w_r_prev = np.asarray(w_r_prev, f32)

    # host-side layout prep (weights + per-core shards)
    W_ihT = np.ascontiguousarray(np.asarray(W_ih, f32)[:, :512].T)
    W_hhT = np.ascontiguousarray(np.asarray(W_hh, f32).T)
    W_pT = np.ascontiguousarray(np.asarray(W_p, f32).T)
    b2 = np.ascontiguousarray(np.broadcast_to(
        (np.asarray(b_ih, f32) + np.asarray(b_hh, f32))[None, :], (128, 2048)))
    bp = np.ascontiguousarray(np.broadcast_to(
        np.asarray(b_p, f32)[None, :], (128, 1028)))
    h0t = np.ascontiguousarray(np.asarray(h0, f32).reshape(4, 128).T)
    c0r = np.asarray(c0, f32).reshape(1, 512)

    rv = np.transpose(np.asarray(read_vectors, f32), (1, 0, 2)).reshape(B, R * D)
    has_rv = bool(np.any(rv))
    xrv = (rv @ np.asarray(W_ih, f32)[:, 512:].T) if has_rv else None

    norm = np.sqrt(np.einsum("bnd,bnd->bn", M_prev, M_prev, dtype=np.float64,
                             optimize=True)).astype(f32)        # [B, N]
    Mn = M_prev / (norm[:, :, None] + 1e-30)
    Mn_bf = _bf16(Mn)

    in_maps = []
    for c in range(NC):
        b0 = c * BS
        sl = slice(b0, b0 + BS)
        mnat = np.ascontiguousarray(
            Mn_bf[sl].reshape(BS, NT, 128, 256).transpose(2, 0, 1, 3))
        mT = np.ascontiguousarray(
            Mn_bf[sl].transpose(0, 2, 1)        # [BS, 256, 2048]
            .reshape(BS, 2, 128, 2048).transpose(1, 2, 0, 3))
        wuT = np.ascontiguousarray(
            w_u_prev[sl].reshape(BS, NT, 128).transpose(2, 0, 1))
        normT = np.ascontiguousarray(
            norm[sl].reshape(BS, NT, 128).transpose(2, 0, 1))
        wrpT = np.ascontiguousarray(
            np.asarray(w_r_prev, f32)[:, sl].transpose(1, 2, 0)  # [BS, N, R]
            .reshape(BS, NT, 128, 4).transpose(2, 0, 1, 3))
        bselT = np.zeros((128, BS), f32)
        bselT[np.arange(b0, b0 + BS), np.arange(BS)] = 1.0
        m = dict(xin=inputs, h0t=h0t, c0=c0r, b2=b2, bp=bp,
                 wihT=W_ihT, whhT=W_hhT, wpT=W_pT, bselT=bselT,
                 mnat=mnat, mT=mT, wuT=wuT, normT=normT, wrpT=wrpT,
                 wu=np.ascontiguousarray(w_u_prev[sl]))
        if has_rv:
            m["xrv"] = np.ascontiguousarray(xrv)
        in_maps.append(m)

    nc = _build_nc(has_rv, stages=int(os.environ.get("MANN_STAGES", "7")))
    if not nc.is_finalized():
        nc.finalize()
    trace = os.environ.get("MANN_TRACE", "0") == "1"
    if trace:
        trace = _ensure_ntff_hook()
    res = run_bass_kernel_spmd(nc, in_maps, core_ids=list(range(NC)),
                               trace=trace,
                               trace_cores=list(range(NC)) if trace else None)
    _LAST_RESULTS["res"] = res

    out = np.concatenate([res.results[c]["out"] for c in range(NC)], axis=0)
    return np.ascontiguousarray(out.astype(f32))
